# revision 26
# baseline (speedup 1.0000x reference)
"""Multi-head causal attention on 8 Trainium2 NeuronCores.

Sharding: core c -> batch b = c // 4, head-group g = c % 4 (4 of 16 heads).
Each core computes its 4 heads' attention and the partial W_O contraction;
the host sums the 4 head-group partials per batch (the reduce of the
tensor-parallel split).

Device-side layout is transpose-free: the host pre-transposes x and the
weights so every matmul contraction lands on the partition axis:
  qT[e,s], kT[e,s]  = W^T-chunk.T @ xT-chunk          (accum over d)
  v[m,he]           = xT-chunk.T @ WvT-chunk          (accum over d)
  sT[m,s]           = kT-slice.T @ qT-block           (scores, transposed)
  pT[m,s]           = exp(sT * 1/sqrt(e))  * mask     (ScalarE + DVE)
  zT[e,s]          += v-slice.T @ pT                  (accum over m)
  den[1,s]         += ones.T @ pT                     (softmax denominator)
  recip             = exp(-ln(den))                   (ScalarE, 2 passes)
  zn[e,s]           = zT * (ones x recip)             (PE outer-prod bcast)
  out[s,d]         += zn-slice.T @ WoT                (accum over heads)

All matmul operands fp16 (full PE rate), accumulation fp32 in PSUM.
"""

import math

import numpy as np

B = 2
S = 2048
D = 2048
H = 16
E = 128
HPC = 4          # heads per core
HE = HPC * E     # 512
NC_CHUNKS = D // 128   # 16 contraction chunks of 128
NBLK = 4         # s-blocks of 512
NMT = S // 128   # 16 m-tiles of 128
SCALE = 1.0 / math.sqrt(E)
N_CORES = 8

_CACHE = {}


def _build_program():
    import concourse.bacc as bacc
    import concourse.mybir as mybir
    import concourse.tile as tile

    f16 = mybir.dt.float16
    f32 = mybir.dt.float32
    Exp = mybir.ActivationFunctionType.Exp
    Ln = mybir.ActivationFunctionType.Ln

    nc = bacc.Bacc("TRN2", target_bir_lowering=False, debug=False,
                   num_devices=N_CORES)

    xT_d = nc.dram_tensor("xT", [D, S], f16, kind="ExternalInput")
    wq_d = nc.dram_tensor("wq", [D, HE], f16, kind="ExternalInput")
    wk_d = nc.dram_tensor("wk", [D, HE], f16, kind="ExternalInput")
    wv_d = nc.dram_tensor("wv", [D, HE], f16, kind="ExternalInput")
    woT_d = nc.dram_tensor("woT", [HE, D], f16, kind="ExternalInput")
    ones32_d = nc.dram_tensor("ones32", [1, 128], f32, kind="ExternalInput")
    masks_d = nc.dram_tensor("masks", [HPC, 128, 512], f16, kind="ExternalInput")
    ones_d = nc.dram_tensor("ones", [128, 129], f16, kind="ExternalInput")
    outp_d = nc.dram_tensor("outp", [S, D], f16, kind="ExternalOutput")

    with tile.TileContext(nc) as tc:
        with (
            tc.tile_pool(name="const", bufs=1) as constp,
            tc.tile_pool(name="qkv", bufs=1) as qkvp,
            tc.tile_pool(name="small", bufs=1) as smallp,
            tc.tile_pool(name="pt", bufs=8) as ptp,
        ):
            woT_sb = constp.tile([128, HPC, D], f16, tag="woT")
            nc.sync.dma_start(
                woT_sb[:], woT_d.rearrange("(c p) d -> p c d", p=128))
            ones_sb = constp.tile([128, 129], f16, tag="ones")
            nc.sync.dma_start(ones_sb[:], ones_d[:])
            onesm = ones_sb[:, 0:1]            # [128, 1] denominator lhsT
            onescol = ones_sb[0:1, 1:129]      # [1, 128] broadcast lhsT

            masks_sb = smallp.tile([128, HPC, 512], f16, tag="masks")
            nc.sync.dma_start(
                masks_sb[:], masks_d.rearrange("r p s -> p r s"))

            qT = [qkvp.tile([128, S], f16, tag=f"qT{h}", name=f"qT{h}")
                  for h in range(HPC)]
            kT = [qkvp.tile([128, S], f16, tag=f"kT{h}", name=f"kT{h}")
                  for h in range(HPC)]
            vt = [qkvp.tile([128, HE], f16, tag=f"v{m}", name=f"v{m}")
                  for m in range(NMT)]

            # ---- Phase B: projections (xT + W streamed through a big pool)
            with (
                tc.tile_pool(name="big", bufs=1) as bigp,
                tc.tile_pool(name="psumB", bufs=4, space="PSUM") as psB,
            ):
                # DMA order tuned so the first projection chain (kT, j=0)
                # can start after ~2MB: wk then the j=0 slice of xT, then
                # the rest streams in behind the compute.
                xT_sb = bigp.tile([128, NC_CHUNKS, S], f16, tag="xT")
                src = xT_d.rearrange("(c p) s -> p c s", p=128)
                w_sb = {}
                for name, dram in (("wq", wq_d), ("wk", wk_d), ("wv", wv_d)):
                    w_sb[name] = bigp.tile([128, NC_CHUNKS, HE], f16,
                                           tag=name, name=name + "_sb")
                wsrc = {name: dram.rearrange("(c p) n -> p c n", p=128)
                        for name, dram in
                        (("wq", wq_d), ("wk", wk_d), ("wv", wv_d))}

                def load_w(name, piece, npieces=2):
                    w = NC_CHUNKS // npieces
                    nc.sync.dma_start(
                        w_sb[name][:, w * piece:w * piece + w, :],
                        wsrc[name][:, w * piece:w * piece + w, :])

                def load_x(j, half):
                    nc.sync.dma_start(
                        xT_sb[:, 8 * half:8 * half + 8,
                              j * 512:(j + 1) * 512],
                        src[:, 8 * half:8 * half + 8,
                            j * 512:(j + 1) * 512])

                # first-needed data in ~0.25MB pieces so no single queue's
                # serial drain (31GB/s) gates the first matmul chain
                for e8 in range(8):
                    nc.sync.dma_start(
                        w_sb["wk"][:, 2 * e8:2 * e8 + 2, :],
                        wsrc["wk"][:, 2 * e8:2 * e8 + 2, :])
                    nc.sync.dma_start(
                        xT_sb[:, 2 * e8:2 * e8 + 2, 0:512],
                        src[:, 2 * e8:2 * e8 + 2, 0:512])
                for q in range(4):
                    load_w("wq", q, 4)
                    load_w("wv", q, 4)
                for j in range(1, NBLK):
                    for half in range(2):
                        load_x(j, half)

                def proj(dst_ap, lhs_of_c, rhs_of_c):
                    ps = psB.tile([128, 512], f32, tag="proj", name="ps")
                    for c in range(NC_CHUNKS):
                        nc.tensor.matmul(
                            ps[:], lhsT=lhs_of_c(c), rhs=rhs_of_c(c),
                            start=(c == 0), stop=(c == NC_CHUNKS - 1))
                    nc.vector.tensor_copy(dst_ap, ps[:])

                for j in range(NBLK):
                    for h in range(HPC):
                        for dst, w in ((kT[h], w_sb["wk"]),
                                       (qT[h], w_sb["wq"])):
                            proj(dst[:, j * 512:(j + 1) * 512],
                                 lambda c, w=w, h=h: w[:, c,
                                                      h * E:(h + 1) * E],
                                 lambda c, j=j: xT_sb[:, c,
                                                      j * 512:(j + 1) * 512])
                    for m in range(4 * j, 4 * j + 4):
                        proj(vt[m][:],
                             lambda c, m=m: xT_sb[:, c,
                                                  m * 128:(m + 1) * 128],
                             lambda c: w_sb["wv"][:, c, :])

            # ---- Phases C+D: attention, normalize, output projection
            with (
                tc.tile_pool(name="post", bufs=1) as postp,
                tc.tile_pool(name="work", bufs=2) as workp,
                tc.tile_pool(name="osb", bufs=4) as osbp,
                tc.tile_pool(name="psO", bufs=2, space="PSUM") as psO,
            ):
                zn = [[None] * NBLK for _ in range(HPC)]

                def emit_out_block(j, pool):
                    for st in range(4):
                        for db in range(4):
                            ops = pool.tile([128, 512], f32, tag="o",
                                            name="ops")
                            for h in range(HPC):
                                nc.tensor.matmul(
                                    ops[:],
                                    lhsT=zn[h][j][:, st * 128:(st + 1) * 128],
                                    rhs=woT_sb[:, h,
                                               db * 512:(db + 1) * 512],
                                    start=(h == 0), stop=(h == HPC - 1))
                            osb = osbp.tile([128, 512], f16, tag="osb",
                                            name="osb")
                            nc.vector.tensor_copy(osb[:], ops[:])
                            row = j * 512 + st * 128
                            nc.sync.dma_start(
                                outp_d[row:row + 128,
                                       db * 512:(db + 1) * 512], osb[:])

                import contextlib
                attn_stack = contextlib.ExitStack()
                psS = attn_stack.enter_context(
                    tc.tile_pool(name="psS", bufs=2, space="PSUM"))
                psZ = attn_stack.enter_context(
                    tc.tile_pool(name="psZ", bufs=2, space="PSUM"))
                psM = attn_stack.enter_context(
                    tc.tile_pool(name="psM", bufs=1, space="PSUM"))
                psB2 = attn_stack.enter_context(
                    tc.tile_pool(name="psB2", bufs=1, space="PSUM"))
                # normalization tail of the previous head, emitted a few
                # chunks into the next head's stream so the in-order PE
                # never waits on the DVE reciprocal chain
                pending = [None]

                def flush_pending():
                    if pending[0] is not None:
                        pending[0]()
                        pending[0] = None

                for j in range(NBLK):
                    for h in range(HPC):
                        zps = psZ.tile([128, 512], f32, tag="z")
                        dps = psM.tile([1, 512], f32, tag="m")
                        nchunks = 4 * j + 4
                        # software pipeline: scores/exp run 2 chunks ahead
                        # of PV/den so the PE never waits on a fresh exp
                        pts = [None] * nchunks
                        cols = [None] * nchunks

                        def emit_score(i):
                            # columns < c0 are fully masked (never read)
                            r = i - 4 * j
                            c0 = 128 * r if r > 0 else 0
                            cols[i] = c0
                            sps = psS.tile([128, 512], f32, tag="s",
                                           name="sps")
                            nc.tensor.matmul(
                                sps[:, c0:512],
                                lhsT=kT[h][:, i * 128:(i + 1) * 128],
                                rhs=qT[h][:, j * 512 + c0:(j + 1) * 512],
                                start=True, stop=True)
                            pt = ptp.tile([128, 512], f16, tag="pt",
                                          name="pt")
                            nc.scalar.activation(pt[:, c0:512],
                                                 sps[:, c0:512], Exp,
                                                 scale=SCALE)
                            if r >= 0:
                                # only the 128-wide diagonal band is
                                # partially masked
                                nc.vector.tensor_mul(
                                    pt[:, c0:c0 + 128], pt[:, c0:c0 + 128],
                                    masks_sb[:, 0, 0:128])
                            pts[i] = pt

                        def emit_pv(i):
                            c0 = cols[i]
                            pt = pts[i]
                            last = (i == nchunks - 1)
                            nc.tensor.matmul(
                                zps[:, c0:512],
                                lhsT=vt[i][:, h * E:(h + 1) * E],
                                rhs=pt[:, c0:512], start=(i == 0), stop=last,
                                skip_group_check=(c0 > 0))
                            nc.tensor.matmul(
                                dps[:, c0:512], lhsT=onesm,
                                rhs=pt[:, c0:512], start=(i == 0), stop=last,
                                skip_group_check=(c0 > 0))
                            pts[i] = None

                        off = min(3, nchunks - 1)
                        for i in range(nchunks):
                            emit_score(i)
                            if i == 2:
                                flush_pending()
                            if i >= off:
                                emit_pv(i - off)
                        for i in range(nchunks - off, nchunks):
                            emit_pv(i)
                        rec32 = workp.tile([1, 512], f32, tag="rec32")
                        nc.vector.reciprocal_approx_fast(rec32[:], dps[:])
                        rec = workp.tile([1, 512], f16, tag="rec")
                        nc.vector.tensor_copy(rec[:], rec32[:])

                        def normalize(h=h, j=j, zps=zps, rec=rec):
                            bps = psB2.tile([128, 512], f32, tag="b")
                            nc.tensor.matmul(
                                bps[:], lhsT=onescol, rhs=rec[:],
                                start=True, stop=True)
                            bsb = workp.tile([128, 512], f16, tag="bsb",
                                             name="bsb")
                            nc.vector.tensor_copy(bsb[:], bps[:])
                            z = postp.tile([128, 512], f16, tag=f"zn{h}_{j}",
                                           name=f"zn{h}_{j}")
                            nc.vector.tensor_mul(z[:], zps[:], bsb[:])
                            zn[h][j] = z

                        pending[0] = normalize

                    # Output projection for this j-block (all 4 heads ready).
                    flush_pending()
                    # The last block is emitted after the attention psum
                    # pools close so it gets a deep pool for the tail.
                    if j < NBLK - 1:
                        emit_out_block(j, psO)
                attn_stack.close()
                with tc.tile_pool(name="psO3", bufs=4,
                                  space="PSUM") as psO3:
                    emit_out_block(NBLK - 1, psO3)

    nc.compile()
    return nc


def _get_nc():
    if "nc" not in _CACHE:
        _CACHE["nc"] = _build_program()
    return _CACHE["nc"]


def _host_inputs(x, W_Q, W_K, W_V, W_O):
    """Per-core input dicts (all fp16, pre-transposed)."""
    masks = np.zeros((HPC, 128, 512), dtype=np.float16)
    cc = np.arange(512)[None, :]
    mm = np.arange(128)[:, None]
    for r in range(HPC):
        masks[r] = (cc >= 128 * r + mm).astype(np.float16)
    ones = np.ones((128, 129), dtype=np.float16)
    ones32 = np.ones((1, 128), dtype=np.float32)

    in_maps = []
    for c in range(N_CORES):
        b, g = divmod(c, 4)
        hs = slice(HPC * g, HPC * g + HPC)
        xT = np.ascontiguousarray(x[b].T).astype(np.float16)
        wq = np.ascontiguousarray(
            W_Q[hs].transpose(2, 0, 1).reshape(D, HE)).astype(np.float16)
        wk = np.ascontiguousarray(
            W_K[hs].transpose(2, 0, 1).reshape(D, HE)).astype(np.float16)
        wv = np.ascontiguousarray(
            W_V[hs].transpose(2, 0, 1).reshape(D, HE)).astype(np.float16)
        woT = np.ascontiguousarray(
            W_O[hs].transpose(0, 2, 1).reshape(HE, D)).astype(np.float16)
        in_maps.append({"xT": xT, "wq": wq, "wk": wk, "wv": wv,
                        "woT": woT, "masks": masks, "ones": ones,
                        "ones32": ones32})
    return in_maps


def _run(in_maps, trace=False, **kw):
    from concourse.bass_utils import run_bass_kernel_spmd
    nc = _get_nc()
    return run_bass_kernel_spmd(nc, in_maps, list(range(N_CORES)),
                                trace=trace, **kw)


def kernel(x, W_Q, W_K, W_V, W_O):
    res = _run(_host_inputs(x, W_Q, W_K, W_V, W_O))
    parts = [np.asarray(res.results[c]["outp"], dtype=np.float32)
             for c in range(N_CORES)]
    out = np.stack([parts[0] + parts[1] + parts[2] + parts[3],
                    parts[4] + parts[5] + parts[6] + parts[7]])
    return out


# revision 30
# speedup vs baseline: 1.0032x; 1.0032x over previous
"""Multi-head causal attention on 8 Trainium2 NeuronCores.

Sharding: core c -> batch b = c // 4, head-group g = c % 4 (4 of 16 heads).
Each core computes its 4 heads' attention and the partial W_O contraction;
the host sums the 4 head-group partials per batch (the reduce of the
tensor-parallel split).

Device-side layout is transpose-free: the host pre-transposes x and the
weights so every matmul contraction lands on the partition axis:
  qT[e,s], kT[e,s]  = W^T-chunk.T @ xT-chunk          (accum over d)
  v[m,he]           = xT-chunk.T @ WvT-chunk          (accum over d)
  sT[m,s]           = kT-slice.T @ qT-block           (scores, transposed)
  pT[m,s]           = exp(sT * 1/sqrt(e))  * mask     (ScalarE + DVE)
  zT[e,s]          += v-slice.T @ pT                  (accum over m)
  den[1,s]         += ones.T @ pT                     (softmax denominator)
  recip             = exp(-ln(den))                   (ScalarE, 2 passes)
  zn[e,s]           = zT * (ones x recip)             (PE outer-prod bcast)
  out[s,d]         += zn-slice.T @ WoT                (accum over heads)

All matmul operands fp16 (full PE rate), accumulation fp32 in PSUM.
"""

import math

import numpy as np

B = 2
S = 2048
D = 2048
H = 16
E = 128
HPC = 4          # heads per core
HE = HPC * E     # 512
NC_CHUNKS = D // 128   # 16 contraction chunks of 128
NBLK = 4         # s-blocks of 512
NMT = S // 128   # 16 m-tiles of 128
SCALE = 1.0 / math.sqrt(E)
N_CORES = 8

_CACHE = {}


def _build_program():
    import concourse.bacc as bacc
    import concourse.mybir as mybir
    import concourse.tile as tile

    f16 = mybir.dt.float16
    f32 = mybir.dt.float32
    Exp = mybir.ActivationFunctionType.Exp
    Ln = mybir.ActivationFunctionType.Ln

    nc = bacc.Bacc("TRN2", target_bir_lowering=False, debug=False,
                   num_devices=N_CORES)

    xT_d = nc.dram_tensor("xT", [D, S], f16, kind="ExternalInput")
    wq_d = nc.dram_tensor("wq", [D, HE], f16, kind="ExternalInput")
    wk_d = nc.dram_tensor("wk", [D, HE], f16, kind="ExternalInput")
    wv_d = nc.dram_tensor("wv", [D, HE], f16, kind="ExternalInput")
    woT_d = nc.dram_tensor("woT", [HE, D], f16, kind="ExternalInput")
    ones32_d = nc.dram_tensor("ones32", [1, 128], f32, kind="ExternalInput")
    masks_d = nc.dram_tensor("masks", [HPC, 128, 512], f16, kind="ExternalInput")
    ones_d = nc.dram_tensor("ones", [128, 129], f16, kind="ExternalInput")
    outp_d = nc.dram_tensor("outp", [S, D], f16, kind="ExternalOutput")

    with tile.TileContext(nc) as tc:
        with (
            tc.tile_pool(name="const", bufs=1) as constp,
            tc.tile_pool(name="qkv", bufs=1) as qkvp,
            tc.tile_pool(name="small", bufs=1) as smallp,
            tc.tile_pool(name="pt", bufs=8) as ptp,
        ):
            woT_sb = constp.tile([128, HPC, D], f16, tag="woT")
            nc.sync.dma_start(
                woT_sb[:], woT_d.rearrange("(c p) d -> p c d", p=128))
            ones_sb = constp.tile([128, 129], f16, tag="ones")
            nc.sync.dma_start(ones_sb[:], ones_d[:])
            onesm = ones_sb[:, 0:1]            # [128, 1] denominator lhsT
            onescol = ones_sb[0:1, 1:129]      # [1, 128] broadcast lhsT

            masks_sb = smallp.tile([128, HPC, 512], f16, tag="masks")
            nc.sync.dma_start(
                masks_sb[:], masks_d.rearrange("r p s -> p r s"))

            qT = [qkvp.tile([128, S], f16, tag=f"qT{h}", name=f"qT{h}")
                  for h in range(HPC)]
            kT = [qkvp.tile([128, S], f16, tag=f"kT{h}", name=f"kT{h}")
                  for h in range(HPC)]
            vt = [qkvp.tile([128, HE], f16, tag=f"v{m}", name=f"v{m}")
                  for m in range(NMT)]

            # ---- Phase B: projections (xT + W streamed through a big pool)
            with (
                tc.tile_pool(name="big", bufs=1) as bigp,
                tc.tile_pool(name="psumB", bufs=4, space="PSUM") as psB,
            ):
                # DMA order tuned so the first projection chain (kT, j=0)
                # can start after ~2MB: wk then the j=0 slice of xT, then
                # the rest streams in behind the compute.
                xT_sb = bigp.tile([128, NC_CHUNKS, S], f16, tag="xT")
                src = xT_d.rearrange("(c p) s -> p c s", p=128)
                w_sb = {}
                for name, dram in (("wq", wq_d), ("wk", wk_d), ("wv", wv_d)):
                    w_sb[name] = bigp.tile([128, NC_CHUNKS, HE], f16,
                                           tag=name, name=name + "_sb")
                wsrc = {name: dram.rearrange("(c p) n -> p c n", p=128)
                        for name, dram in
                        (("wq", wq_d), ("wk", wk_d), ("wv", wv_d))}

                def load_w(name, piece, npieces=2):
                    w = NC_CHUNKS // npieces
                    nc.sync.dma_start(
                        w_sb[name][:, w * piece:w * piece + w, :],
                        wsrc[name][:, w * piece:w * piece + w, :])

                def load_x(j, half):
                    nc.sync.dma_start(
                        xT_sb[:, 8 * half:8 * half + 8,
                              j * 512:(j + 1) * 512],
                        src[:, 8 * half:8 * half + 8,
                            j * 512:(j + 1) * 512])

                # first-needed data in small pieces so no single queue's
                # serial drain (31GB/s) gates the first matmul chain
                for c in range(4):
                    nc.sync.dma_start(w_sb["wk"][:, c:c + 1, :],
                                      wsrc["wk"][:, c:c + 1, :])
                    nc.sync.dma_start(xT_sb[:, c:c + 1, 0:512],
                                      src[:, c:c + 1, 0:512])
                for e6 in range(2, 8):
                    nc.sync.dma_start(
                        w_sb["wk"][:, 2 * e6:2 * e6 + 2, :],
                        wsrc["wk"][:, 2 * e6:2 * e6 + 2, :])
                    nc.sync.dma_start(
                        xT_sb[:, 2 * e6:2 * e6 + 2, 0:512],
                        src[:, 2 * e6:2 * e6 + 2, 0:512])
                for q in range(4):
                    load_w("wq", q, 4)
                    load_w("wv", q, 4)
                for j in range(1, NBLK):
                    for half in range(2):
                        load_x(j, half)

                def proj(dst_ap, lhs_of_c, rhs_of_c):
                    ps = psB.tile([128, 512], f32, tag="proj", name="ps")
                    for c in range(NC_CHUNKS):
                        nc.tensor.matmul(
                            ps[:], lhsT=lhs_of_c(c), rhs=rhs_of_c(c),
                            start=(c == 0), stop=(c == NC_CHUNKS - 1))
                    nc.vector.tensor_copy(dst_ap, ps[:])

                for j in range(NBLK):
                    for h in range(HPC):
                        for dst, w in ((kT[h], w_sb["wk"]),
                                       (qT[h], w_sb["wq"])):
                            proj(dst[:, j * 512:(j + 1) * 512],
                                 lambda c, w=w, h=h: w[:, c,
                                                      h * E:(h + 1) * E],
                                 lambda c, j=j: xT_sb[:, c,
                                                      j * 512:(j + 1) * 512])
                    for m in range(4 * j, 4 * j + 4):
                        proj(vt[m][:],
                             lambda c, m=m: xT_sb[:, c,
                                                  m * 128:(m + 1) * 128],
                             lambda c: w_sb["wv"][:, c, :])

            # ---- Phases C+D: attention, normalize, output projection
            with (
                tc.tile_pool(name="post", bufs=1) as postp,
                tc.tile_pool(name="work", bufs=2) as workp,
                tc.tile_pool(name="osb", bufs=4) as osbp,
                tc.tile_pool(name="psO", bufs=2, space="PSUM") as psO,
            ):
                zn = [[None] * NBLK for _ in range(HPC)]

                def emit_out_block(j, pool):
                    for st in range(4):
                        for db in range(4):
                            ops = pool.tile([128, 512], f32, tag="o",
                                            name="ops")
                            for h in range(HPC):
                                nc.tensor.matmul(
                                    ops[:],
                                    lhsT=zn[h][j][:, st * 128:(st + 1) * 128],
                                    rhs=woT_sb[:, h,
                                               db * 512:(db + 1) * 512],
                                    start=(h == 0), stop=(h == HPC - 1))
                            osb = osbp.tile([128, 512], f16, tag="osb",
                                            name="osb")
                            nc.vector.tensor_copy(osb[:], ops[:])
                            row = j * 512 + st * 128
                            nc.sync.dma_start(
                                outp_d[row:row + 128,
                                       db * 512:(db + 1) * 512], osb[:])

                import contextlib
                attn_stack = contextlib.ExitStack()
                psS = attn_stack.enter_context(
                    tc.tile_pool(name="psS", bufs=3, space="PSUM"))
                psZ = attn_stack.enter_context(
                    tc.tile_pool(name="psZ", bufs=2, space="PSUM"))
                psM = attn_stack.enter_context(
                    tc.tile_pool(name="psM", bufs=1, space="PSUM"))
                for j in range(NBLK):
                    for h in range(HPC):
                        zps = psZ.tile([128, 512], f32, tag="z")
                        dps = psM.tile([1, 512], f32, tag="m")
                        nchunks = 4 * j + 4
                        # software pipeline: scores/exp run 2 chunks ahead
                        # of PV/den so the PE never waits on a fresh exp
                        pts = [None] * nchunks
                        cols = [None] * nchunks

                        def emit_score(i):
                            # columns < c0 are fully masked (never read)
                            r = i - 4 * j
                            c0 = 128 * r if r > 0 else 0
                            cols[i] = c0
                            sps = psS.tile([128, 512], f32, tag="s",
                                           name="sps")
                            nc.tensor.matmul(
                                sps[:, c0:512],
                                lhsT=kT[h][:, i * 128:(i + 1) * 128],
                                rhs=qT[h][:, j * 512 + c0:(j + 1) * 512],
                                start=True, stop=True)
                            pt = ptp.tile([128, 512], f16, tag="pt",
                                          name="pt")
                            nc.scalar.activation(pt[:, c0:512],
                                                 sps[:, c0:512], Exp,
                                                 scale=SCALE)
                            if r >= 0:
                                # only the 128-wide diagonal band is
                                # partially masked
                                nc.vector.tensor_mul(
                                    pt[:, c0:c0 + 128], pt[:, c0:c0 + 128],
                                    masks_sb[:, 0, 0:128])
                            pts[i] = pt

                        def emit_pv(i):
                            c0 = cols[i]
                            pt = pts[i]
                            last = (i == nchunks - 1)
                            nc.tensor.matmul(
                                zps[:, c0:512],
                                lhsT=vt[i][:, h * E:(h + 1) * E],
                                rhs=pt[:, c0:512], start=(i == 0), stop=last,
                                skip_group_check=(c0 > 0))
                            nc.tensor.matmul(
                                dps[:, c0:512], lhsT=onesm,
                                rhs=pt[:, c0:512], start=(i == 0), stop=last,
                                skip_group_check=(c0 > 0))
                            pts[i] = None

                        off = min(3, nchunks - 1)
                        for i in range(nchunks):
                            emit_score(i)
                            if i >= off:
                                emit_pv(i - off)
                        for i in range(nchunks - off, nchunks):
                            emit_pv(i)
                        rec32 = workp.tile([1, 512], f32, tag="rec32")
                        nc.vector.reciprocal_approx_fast(rec32[:], dps[:])
                        rec = workp.tile([1, 512], f16, tag="rec")
                        nc.vector.tensor_copy(rec[:], rec32[:])
                        bps = psM.tile([128, 512], f32, tag="m")
                        nc.tensor.matmul(bps[:], lhsT=onescol, rhs=rec[:],
                                         start=True, stop=True)
                        bsb = workp.tile([128, 512], f16, tag="bsb")
                        nc.vector.tensor_copy(bsb[:], bps[:])
                        z = postp.tile([128, 512], f16, tag=f"zn{h}_{j}",
                                       name=f"zn{h}_{j}")
                        nc.vector.tensor_mul(z[:], zps[:], bsb[:])
                        zn[h][j] = z

                    # Output projection for this j-block (all 4 heads ready).
                    # The last block is emitted after the attention psum
                    # pools close so it gets a deep pool for the tail.
                    if j < NBLK - 1:
                        emit_out_block(j, psO)
                attn_stack.close()
                with tc.tile_pool(name="psO3", bufs=4,
                                  space="PSUM") as psO3:
                    emit_out_block(NBLK - 1, psO3)

    nc.compile()
    return nc


def _get_nc():
    if "nc" not in _CACHE:
        _CACHE["nc"] = _build_program()
    return _CACHE["nc"]


def _host_inputs(x, W_Q, W_K, W_V, W_O):
    """Per-core input dicts (all fp16, pre-transposed)."""
    masks = np.zeros((HPC, 128, 512), dtype=np.float16)
    cc = np.arange(512)[None, :]
    mm = np.arange(128)[:, None]
    for r in range(HPC):
        masks[r] = (cc >= 128 * r + mm).astype(np.float16)
    ones = np.ones((128, 129), dtype=np.float16)
    ones32 = np.ones((1, 128), dtype=np.float32)

    in_maps = []
    for c in range(N_CORES):
        b, g = divmod(c, 4)
        hs = slice(HPC * g, HPC * g + HPC)
        xT = np.ascontiguousarray(x[b].T).astype(np.float16)
        wq = np.ascontiguousarray(
            W_Q[hs].transpose(2, 0, 1).reshape(D, HE)).astype(np.float16)
        wk = np.ascontiguousarray(
            W_K[hs].transpose(2, 0, 1).reshape(D, HE)).astype(np.float16)
        wv = np.ascontiguousarray(
            W_V[hs].transpose(2, 0, 1).reshape(D, HE)).astype(np.float16)
        woT = np.ascontiguousarray(
            W_O[hs].transpose(0, 2, 1).reshape(HE, D)).astype(np.float16)
        in_maps.append({"xT": xT, "wq": wq, "wk": wk, "wv": wv,
                        "woT": woT, "masks": masks, "ones": ones,
                        "ones32": ones32})
    return in_maps


def _run(in_maps, trace=False, **kw):
    from concourse.bass_utils import run_bass_kernel_spmd
    nc = _get_nc()
    return run_bass_kernel_spmd(nc, in_maps, list(range(N_CORES)),
                                trace=trace, **kw)


def kernel(x, W_Q, W_K, W_V, W_O):
    res = _run(_host_inputs(x, W_Q, W_K, W_V, W_O))
    parts = [np.asarray(res.results[c]["outp"], dtype=np.float32)
             for c in range(N_CORES)]
    out = np.stack([parts[0] + parts[1] + parts[2] + parts[3],
                    parts[4] + parts[5] + parts[6] + parts[7]])
    return out
